# revision 32
# baseline (speedup 1.0000x reference)
"""Trainium2 Bass kernel for ConsistentSelfAttentionTile.

Reference semantics: T=449 overlapping 64-token tiles; each tile attends to
352 KV tokens = 288 sampled (from a 9x replication of the tile) + the tile
itself; outputs overlap-add, then divide by overlap counts.

Algebraic collapse (verified ~1.3e-3 rel vs the jax reference on CPU):
  * rep[:, idx, :] == tile[:, idx % 64, :], so the sampled KV tokens are tile
    rows with integer multiplicities m_t[w] = 1 + #{s : idx[t,s] % 64 == w}.
  * Per-tile Q/K/V are slices of the full-sequence projections, so all
    per-tile 64x64 score blocks are diagonal blocks of one banded 512x512
    score matrix S = Q K^T (band |i-j| <= 63).
  * With E = exp(S - 20), Cm[j,t] = m_t[j-t] (banded), the full tile-softmax
    + overlap-add + count-divide collapses to
        Z = Cm^T E;  W = bandmask/(counts * Z);  U = Cm W;  out = (E*U)^T V
    computed entirely in the transposed [j, r] layout: S' = K Q^T is emitted
    directly by the PE (no E transposes), and a constant exp bias replaces
    the per-row max (softmax is shift-invariant; bf16/fp32 cover the range).
  * bk drops exactly: it scales each column's E and 1/Z by canceling factors.

Sharding: 8 cores = 2 batches x 4 row-chunks of 128 output rows. Each core
computes its 128 rows end-to-end from a 256-column band of the input (no
cross-core communication); host slices/pads inputs and concatenates outputs.

Schedule notes (all tuned against perfetto traces; 37.9us -> ~31us):
  * Everything ships fp16 (counts/Cm as bf16 bits: small ints, exact); all
    matmuls accumulate in fp32 PSUM. Score-path quantization error ~fp16 is
    ~2.5e-3 absolute on scores -> ~1.3e-3 output rel err, far under 2e-2.
  * A PSUM accumulation region must fully close (stop=True) before another
    region in the SAME bank issues start=True: start clears has_written
    bank-wide, silently dropping earlier regions' first contribution.
  * Input DMA: few LARGE pieces. An HWDGE ring leaves a ~2us gap between
    FIFO pieces (descriptors for piece N+1 wait on piece N's completion
    receipt), so 0.25MB pieces run a ring at ~1/3 duty cycle. Three queues
    (sync ring / scalar ring / gpsimd SWDGE) carry 2 pieces each, ordered
    by first use; the bias row ships as a single-partition 2.3KB piece
    instead of a mostly-zero [128, 1152] block.
  * ~8 throwaway warmup matmuls (>=3.6us busy) run while the first piece
    lands so the PE's HAM clock gate reaches 8/8 before the real stream.
  * PSUM->SBUF drains are split between Vector and Scalar (GPSIMD cannot
    read PSUM); reciprocal uses vector.reciprocal_approx_fast (~350ns vs
    ~1us for the exact DVE reciprocal; Z in [1e-13,1e11] is safe).
  * No-sync dependency edges pin the PE's static order to
    Z -> V(jc0) -> U -> V(jc1) -> out so V's twelve matmuls fill the PE
    idle windows while Vector/ACT run the recip/W and A stages.
  * The TileContext exit is instruction-free (bookkeeping only) and
    Bacc.reset()'s full-pool sem wipe is neutered: the walrus NEFF
    epilogue already barriers, drains, and sweeps every semaphore, so the
    in-body duplicates (~5us) are dead weight. Output-DMA completion is
    covered by that same ~8us epilogue (validated by double-execution).
"""

import os
import sys

import numpy as np

try:
    import ml_dtypes
except ImportError:
    ml_dtypes = None

for _p in ("/opt/trn_rl_repo",):
    if _p not in sys.path and os.path.isdir(_p):
        sys.path.insert(0, _p)

B, N, C, W = 2, 512, 512, 64
T = N - W + 1          # 449 tiles
RCH = 128              # output rows per core
NCORES = 8
BAND = 256             # per-core j/t band width (columns [r0-64, r0+192))
KC = C // 128          # 4 contraction chunks
JC = BAND // 128       # 2 band chunks
EXP_BIAS = -20.0       # constant softmax shift (shift-invariant; keeps
                       # exp args in a comfortably representable range)
N_WARM = 8             # PE warmup matmuls (HAM un-throttle)

# blob16 layout (2-byte elements per partition; fp16 except the bf16 Cm
# segments, which are bitcast views)
OFF_XT = 0                        # [128, 4, 256] fp16
OFF_WQT = OFF_XT + KC * BAND      # [128, 4, 512] fp16
OFF_WKT = OFF_WQT + KC * C        # [128, 4, 512] fp16
OFF_WVT = OFF_WKT + KC * C        # [128, 4, 512] fp16
OFF_CM = OFF_WVT + KC * C         # [128, 2, 256] bf16 bits
OFF_CMT = OFF_CM + JC * BAND      # [128, 2, 256] bf16 bits
OFF_MW = OFF_CMT + JC * BAND      # [128, 2, 128] fp16 (bandmask/counts)
OFF_MISC = OFF_MW + JC * RCH      # partition 0 only: bq[512] bv[512] ones[128]
MISC_LEN = 2 * C + 128
F16 = OFF_MISC + MISC_LEN

_CACHE = {}


def _slim_drain_and_barrier(self, tick_clock, wait_clock):
    """Instruction-free TileContext exit. The stock exit emits a global
    drain + barrier + sem clears + barrier (~3us); but this program's
    epilogue already contains Bacc.reset()'s two all-engine barriers and
    the walrus NEFF teardown (per-engine drains + a full semaphore sweep
    that zeroes every sem below 256), so everything the stock exit does is
    re-done later anyway. The output DMAs' completion receipt (~2us after
    last byte) lands well inside that ~8us epilogue, so nothing needs to
    block on the DMA clock either. Only the allocator bookkeeping stays."""
    popped = self.nc._tile_sem_poison_stack.pop()
    assert popped is self._sem_poison
    for h in self.sems.allocated().values():
        self.nc.release_semaphore(h)


def _build_program():
    import concourse.bacc as bacc
    import concourse.mybir as mybir
    import concourse.tile as tile

    fp16 = mybir.dt.float16
    # Bass's preamble ends with a full all-engine barrier (drains + EVSEM,
    # ~3-5us with the PE's first-IRAM-block stall). Our kernel never reads
    # the preamble's const APs and all real cross-engine deps are Tile
    # semaphores, so skip it: engines start independently and the input DMA
    # issues ~5us earlier.
    orig_aeb = bacc.Bacc.all_engine_barrier

    def _noop_aeb(self, *, sem_only=False):
        return None

    bacc.Bacc.all_engine_barrier = _noop_aeb
    try:
        nc = bacc.Bacc("TRN2", target_bir_lowering=False, debug=False)
    finally:
        bacc.Bacc.all_engine_barrier = orig_aeb

    b16_d = nc.declare_dram_parameter("blob16", [128, F16], fp16, isOutput=False)
    out_d = nc.declare_dram_parameter("out", [RCH, C], mybir.dt.float32,
                                      isOutput=True)

    orig_dab = tile.TileContext._drain_and_barrier
    tile.TileContext._drain_and_barrier = _slim_drain_and_barrier
    try:
        _emit_body(nc, tile, mybir, b16_d, out_d)
    finally:
        tile.TileContext._drain_and_barrier = orig_dab

    # compile() emits Bacc.reset() -- the BSP re-entry block -- whose
    # gpsimd.sem_clear(range(3, 256)) lowers to ~250 per-sem EVENT_SEMAPHORE
    # resets spread over 5 engines (~7.5us of pure epilogue, inside the
    # measured window). Every sem this program ever increments is already
    # zeroed by the Tile drain (tile sems) or is self-balancing (barrier /
    # HWDGE-drain protocol sems), so the wipe is redundant: swap in a
    # gpsimd proxy that drops dma_reset/sem_clear during compile.
    class _GpsimdNoResetProxy:
        def __init__(self, real):
            object.__setattr__(self, "_real", real)

        def __getattr__(self, n):
            return getattr(self._real, n)

        def dma_reset(self, semaphore_range=None):
            return None

        def sem_clear(self, sem):
            return None

    real_gpsimd = nc.gpsimd
    nc.gpsimd = _GpsimdNoResetProxy(real_gpsimd)
    try:
        nc.compile()
    finally:
        nc.gpsimd = real_gpsimd
    return nc


def _emit_body(nc, tile, mybir, b16_d, out_d):
    fp32 = mybir.dt.float32
    fp16 = mybir.dt.float16
    bf16 = mybir.dt.bfloat16
    AF = mybir.ActivationFunctionType

    with tile.TileContext(nc) as tc:
        with (
            tc.tile_pool(name="consts", bufs=1) as consts,
            tc.tile_pool(name="work", bufs=1) as work,
            tc.tile_pool(name="psum", bufs=1, space="PSUM") as psum,
        ):
            b16 = consts.tile([128, F16], fp16)

            # ---- PE warmup: un-throttle HAM while the first DMAs land ----
            warm = work.tile([128, 512], fp16)
            nc.gpsimd.memset(warm[:], 0.0)
            ebias = work.tile([128, 1], fp32)
            nc.gpsimd.memset(ebias[:], EXP_BIAS)

            # ---- input DMA: few LARGE pieces over three queues (2 HWDGE
            # rings + 1 SWDGE). A ring leaves a ~2us gap between FIFO
            # pieces (next piece's descriptors wait for the previous
            # completion receipt), so many small pieces run the ring at
            # ~1/3 duty cycle; 1-2 big pieces per queue keep it streaming ----
            sync_pieces = [
                (OFF_WKT, OFF_WKT + KC * C),              # wkt     0.5MB
                (OFF_WVT, OFF_WVT + KC * C),              # wvt     0.5MB
            ]
            scalar_pieces = [
                (OFF_XT, OFF_WQT + 2 * C),                # xt+wqt m01 0.5MB
                (OFF_WQT + 2 * C, OFF_WQT + KC * C),      # wqt m23 0.25MB
            ]
            gpsimd_pieces = [
                None,                                     # misc (1 partition)
                (OFF_CM, OFF_MISC),                       # cm+cmt+mw 0.31MB
            ]
            for eng, pieces in ((nc.sync, sync_pieces),
                                (nc.scalar, scalar_pieces),
                                (nc.gpsimd, gpsimd_pieces)):
                for p in pieces:
                    if p is None:
                        eng.dma_start(
                            out=b16[0:1, OFF_MISC:OFF_MISC + MISC_LEN],
                            in_=b16_d[0:1, OFF_MISC:OFF_MISC + MISC_LEN],
                        )
                    else:
                        a, b = p
                        eng.dma_start(out=b16[:, a:b], in_=b16_d[:, a:b])

            xt_sb = b16[:, OFF_XT:OFF_XT + KC * BAND].rearrange(
                "p (k j) -> p k j", k=KC)
            # wq/wk ship m-major ([m][k][128] cols) so one DMA piece holds
            # two complete output chunks; wv stays k-major (full-N rhs)
            wqt_sb = b16[:, OFF_WQT:OFF_WQT + KC * C].rearrange(
                "p (m x) -> p m x", m=KC)
            wkt_sb = b16[:, OFF_WKT:OFF_WKT + KC * C].rearrange(
                "p (m x) -> p m x", m=KC)
            wvt_sb = b16[:, OFF_WVT:OFF_WVT + KC * C].rearrange(
                "p (k j) -> p k j", k=KC)
            cm_sb = b16[:, OFF_CM:OFF_CM + JC * BAND].bitcast(bf16).rearrange(
                "p (k t) -> p k t", k=JC)
            cmt_sb = b16[:, OFF_CMT:OFF_CMT + JC * BAND].bitcast(
                bf16).rearrange("p (k j) -> p k j", k=JC)
            mw_sb = b16[:, OFF_MW:OFF_MW + JC * RCH].rearrange(
                "p (k r) -> p k r", k=JC)
            bqr = b16[0:1, OFF_MISC:OFF_MISC + C]
            bvr = b16[0:1, OFF_MISC + C:OFF_MISC + 2 * C]
            ones1 = b16[0:1, OFF_MISC + 2 * C:OFF_MISC + 2 * C + 128]

            # PSUM plan (8 banks):
            #   qt [128,512] | kt [128,1024] | v [128,1024] | s [128,256]
            #   (S' jc0/jc1) | zu [128,512] (Z0 Z1 U0 U1) | o [128,512]
            #   (warmup matmuls park their dead results in o's first half)
            ps_qt = psum.tile([128, C], fp32, tag="ps_qt")
            ps_kt = psum.tile([128, KC * BAND], fp32, tag="ps_kt")
            ps_v = psum.tile([128, JC * C], fp32, tag="ps_v")
            ps_s = psum.tile([128, JC * RCH], fp32, tag="ps_s")
            ps_zu = psum.tile([128, 512], fp32, tag="ps_zu")
            ps_o = psum.tile([128, C], fp32, tag="ps_o")

            for i in range(N_WARM):
                nc.tensor.matmul(
                    ps_o[:], lhsT=warm[:, 0:128], rhs=warm[:],
                    start=True, stop=True,
                )

            # ---- QT[c,r] = Wq x_r^T + bq ----
            # NB: each accumulation region must fully close (stop=True)
            # before the next region's start=True in the same PSUM bank --
            # start clears has_written BANK-wide, so interleaving regions
            # silently drops the first contribution of all but the last.
            for m in range(KC):
                for k in range(KC):
                    nc.tensor.matmul(
                        ps_qt[:, m * 128:(m + 1) * 128],
                        lhsT=wqt_sb[:, m, k * 128:(k + 1) * 128],
                        rhs=xt_sb[:, k, 64:64 + RCH],
                        start=(k == 0), stop=False,
                    )
                nc.tensor.matmul(
                    ps_qt[:, m * 128:(m + 1) * 128],
                    lhsT=bqr[:, m * 128:(m + 1) * 128],
                    rhs=ones1, start=False, stop=True,
                )
            qt_sb = work.tile([128, KC * RCH], fp16)
            nc.vector.tensor_copy(out=qt_sb[:], in_=ps_qt)

            # ---- KT[c,j] = Wk x_band^T (bk dropped: softmax-invariant) ----
            for m in range(KC):
                for k in range(KC):
                    nc.tensor.matmul(
                        ps_kt[:, m * BAND:(m + 1) * BAND],
                        lhsT=wkt_sb[:, m, k * 128:(k + 1) * 128],
                        rhs=xt_sb[:, k, :],
                        start=(k == 0), stop=(k == KC - 1),
                    )
            kt_sb = work.tile([128, KC * BAND], fp16)
            nc.vector.tensor_copy(out=kt_sb[:, 0:512], in_=ps_kt[:, 0:512])
            nc.scalar.copy(out=kt_sb[:, 512:1024], in_=ps_kt[:, 512:1024])

            # ---- S'[j,r] = K Q^T, emitted transposed by the PE ----
            for jc in range(JC):
                for m in range(KC):
                    nc.tensor.matmul(
                        ps_s[:, jc * 128:(jc + 1) * 128],
                        lhsT=kt_sb[:, m * BAND + jc * 128:
                                   m * BAND + (jc + 1) * 128],
                        rhs=qt_sb[:, m * RCH:(m + 1) * RCH],
                        start=(m == 0), stop=(m == KC - 1),
                    )
            e_sb = work.tile([128, JC, RCH], bf16)
            for jc in range(JC):
                nc.scalar.activation(
                    out=e_sb[:, jc, :],
                    in_=ps_s[:, jc * 128:(jc + 1) * 128],
                    func=AF.Exp, bias=ebias[:], scale=1.0,
                )

            # ---- V[j,c] = x_band Wv^T + bv ----
            # V is off the critical path (only the final out matmuls read
            # it); no-sync edges below slot its two jc chunks into the PE
            # idle windows of the softmax chain (Z->recip/W and U->A).
            v_first = [None, None]
            v_last = [None, None]
            for jc in range(JC):
                for k in range(KC):
                    mmi = nc.tensor.matmul(
                        ps_v[:, jc * C:(jc + 1) * C],
                        lhsT=xt_sb[:, k, jc * 128:(jc + 1) * 128],
                        rhs=wvt_sb[:, k, :],
                        start=(k == 0), stop=False,
                    )
                    if k == 0:
                        v_first[jc] = mmi
                v_last[jc] = nc.tensor.matmul(
                    ps_v[:, jc * C:(jc + 1) * C],
                    lhsT=ones1, rhs=bvr, start=False, stop=True,
                )
            v_sb = work.tile([128, JC * C], fp16)
            nc.scalar.copy(out=v_sb[:, 0:C], in_=ps_v[:, 0:C])
            nc.scalar.copy(out=v_sb[:, C:2 * C], in_=ps_v[:, C:2 * C])

            # ---- Z[t,r] = Cm^T E;  W = maskw / Z ----
            w_sb = work.tile([128, JC, RCH], bf16)
            z_last = None
            for tch in range(JC):
                zslc = ps_zu[:, tch * 128:(tch + 1) * 128]
                for jc in range(JC):
                    z_last = nc.tensor.matmul(
                        zslc,
                        lhsT=cm_sb[:, jc, tch * 128:(tch + 1) * 128],
                        rhs=e_sb[:, jc, :],
                        start=(jc == 0), stop=(jc == JC - 1),
                    )
                r = work.tile([128, RCH], fp32, tag="rz", bufs=2)
                nc.vector.reciprocal_approx_fast(out=r[:], in_=zslc)
                nc.vector.tensor_mul(w_sb[:, tch, :], r, mw_sb[:, tch, :])

            # ---- U[j,r] = Cm W;  A = E * U ----
            a_sb = work.tile([128, JC, RCH], fp16)
            u_first = None
            u_last = None
            for jc in range(JC):
                uslc = ps_zu[:, 256 + jc * 128:256 + (jc + 1) * 128]
                for tch in range(JC):
                    mmi = nc.tensor.matmul(
                        uslc,
                        lhsT=cmt_sb[:, tch, jc * 128:(jc + 1) * 128],
                        rhs=w_sb[:, tch, :],
                        start=(tch == 0), stop=(tch == JC - 1),
                    )
                    if u_first is None:
                        u_first = mmi
                    u_last = mmi
                nc.vector.tensor_mul(a_sb[:, jc, :], uslc, e_sb[:, jc, :])

            # ---- out[r,c] = A^T V, two column halves overlap the DMA ----
            o_sb = work.tile([128, C], fp32)
            out_first = None
            for h in range(2):
                oslc = ps_o[:, h * 256:(h + 1) * 256]
                for jc in range(JC):
                    mmi = nc.tensor.matmul(
                        oslc,
                        lhsT=a_sb[:, jc, :],
                        rhs=v_sb[:, jc * C + h * 256:jc * C + (h + 1) * 256],
                        start=(jc == 0), stop=(jc == JC - 1),
                    )
                    if out_first is None:
                        out_first = mmi
                if h == 0:
                    nc.vector.tensor_copy(
                        out=o_sb[:, h * 256:(h + 1) * 256], in_=oslc)
                else:
                    nc.scalar.copy(
                        out=o_sb[:, h * 256:(h + 1) * 256], in_=oslc)
                eng = nc.sync if h == 0 else nc.scalar
                eng.dma_start(out=out_d[:, h * 256:(h + 1) * 256],
                              in_=o_sb[:, h * 256:(h + 1) * 256])

            # static PE order: Z -> V(jc0) -> U -> V(jc1) -> out, so V's
            # dozen matmuls fill the PE idle windows while the Vector/ACT
            # engines run the recip/W and A stages of the softmax chain
            from concourse.tile_rust import add_dep_helper
            add_dep_helper(v_first[0].ins, z_last.ins, False,
                           "static order: V jc0 after Z")
            add_dep_helper(u_first.ins, v_last[0].ins, False,
                           "static order: U after V jc0")
            add_dep_helper(v_first[1].ins, u_last.ins, False,
                           "static order: V jc1 after U")
            add_dep_helper(out_first.ins, v_last[1].ins, False,
                           "static order: out after V jc1")


def _pack128(arr):
    """[n*128, f] row-chunked -> [128, n*f] (chunk-major along free axis)."""
    n = arr.shape[0] // 128
    return np.ascontiguousarray(
        arr.reshape(n, 128, -1).transpose(1, 0, 2).reshape(128, -1)
    )


def _host_prep(image_features, Wq, bq, Wk, bk, Wv, bv, sample_idx):
    """Build the 8 per-core input blobs (pure index/layout work)."""
    x = np.asarray(image_features, np.float32)
    sample_idx = np.asarray(sample_idx)

    # per-tile multiplicities -> banded count matrix Cm[j, t] = m_t[j - t]
    mod = (sample_idx % W).astype(np.int64)                  # [T, S]
    m = np.zeros((T, W), np.float32)
    np.add.at(m, (np.arange(T)[:, None], mod), 1.0)
    m += 1.0
    Cm = np.zeros((N, N), np.float32)
    rows = np.arange(T)
    for w in range(W):
        Cm[rows + w, rows] = m[:, w]

    pos = np.arange(N)
    counts = (np.minimum(pos, N - W) - np.maximum(pos - W + 1, 0) + 1)

    # padded versions for uniform band slicing
    XTp = np.zeros((B, C, N + 2 * 64), np.float16)
    for b in range(B):
        XTp[b, :, 64:64 + N] = x[b].T.astype(np.float16)
    Cmp = np.zeros((N + 2 * 64, N + 2 * 64), np.float32)
    Cmp[64:64 + N, 64:64 + N] = Cm

    def _pack_mmajor(w):
        """Wq-like [cout, cin] -> [128, (m, k, 128)] columns: m-chunk-major
        so one contiguous DMA piece carries complete output chunks."""
        p = _pack128(np.asarray(w, np.float32).T.astype(np.float16))
        return np.ascontiguousarray(
            p.reshape(128, KC, KC, 128).transpose(0, 2, 1, 3).reshape(128, -1)
        )

    wqt_p = _pack_mmajor(Wq)
    wkt_p = _pack_mmajor(Wk)
    wvt_p = _pack128(np.asarray(Wv, np.float32).T.astype(np.float16))

    in_maps = []
    for core in range(NCORES):
        b, rc = divmod(core, NCORES // B)
        r0 = rc * RCH
        xt = XTp[b, :, r0:r0 + BAND]
        cm = np.ascontiguousarray(Cmp[r0:r0 + BAND, r0:r0 + BAND])
        # all-zero columns (padded t) would give Z=0 -> 1/0*mask = NaN on
        # device; a diagonal 1 keeps Z finite there and is masked out of W
        zero_cols = ~cm.any(axis=0)
        cm[zero_cols, zero_cols] = 1.0
        tl = np.arange(BAND)
        rl = np.arange(RCH)
        tg = r0 - 64 + tl
        rg = r0 + rl
        d = rg[None, :] - tg[:, None]
        valid = (d >= 0) & (d <= W - 1) & (tg[:, None] >= 0) & (tg[:, None] <= T - 1)
        maskw = np.where(
            valid, 1.0 / counts[rg][None, :], 0.0
        ).astype(np.float16)                                 # [t 256, r 128]

        b16 = np.zeros((128, F16), np.float16)
        b16[:, OFF_XT:OFF_XT + KC * BAND] = _pack128(xt)
        b16[:, OFF_WQT:OFF_WQT + KC * C] = wqt_p
        b16[:, OFF_WKT:OFF_WKT + KC * C] = wkt_p
        b16[:, OFF_WVT:OFF_WVT + KC * C] = wvt_p
        b16[:, OFF_MW:OFF_MW + JC * RCH] = _pack128(maskw)
        b16[0, OFF_MISC:OFF_MISC + C] = np.asarray(bq, np.float32)
        b16[0, OFF_MISC + C:OFF_MISC + 2 * C] = np.asarray(bv, np.float32)
        b16[0, OFF_MISC + 2 * C:OFF_MISC + 2 * C + 128] = 1.0
        # Cm segments carry bf16 bits (count ints are exact in bf16);
        # written through a uint16 view of the fp16 buffer
        b16v = b16.view(np.uint16)
        b16v[:, OFF_CM:OFF_CM + JC * BAND] = _pack128(
            cm.astype(ml_dtypes.bfloat16)).view(np.uint16)
        b16v[:, OFF_CMT:OFF_CMT + JC * BAND] = _pack128(
            np.ascontiguousarray(cm.T).astype(ml_dtypes.bfloat16)
        ).view(np.uint16)
        in_maps.append({"blob16": b16})
    return in_maps


def run_on_cores(in_maps, trace=False, trace_cores=None):
    from concourse.bass_utils import run_bass_kernel_spmd

    if "nc" not in _CACHE:
        _CACHE["nc"] = _build_program()
    nc = _CACHE["nc"]
    return run_bass_kernel_spmd(
        nc, in_maps, list(range(NCORES)), trace=trace,
        trace_cores=(trace_cores or [0]) if trace else None,
    )


def kernel(image_features, Wq, bq, Wk, bk, Wv, bv, sample_idx):
    in_maps = _host_prep(image_features, Wq, bq, Wk, bk, Wv, bv, sample_idx)
    res = run_on_cores(in_maps, trace=False)
    out = np.empty((B, N, C), np.float32)
    for core in range(NCORES):
        b, rc = divmod(core, NCORES // B)
        out[b, rc * RCH:(rc + 1) * RCH, :] = res.results[core]["out"]
    return out


# revision 33
# speedup vs baseline: 1.0686x; 1.0686x over previous
"""Trainium2 Bass kernel for ConsistentSelfAttentionTile.

Reference semantics: T=449 overlapping 64-token tiles; each tile attends to
352 KV tokens = 288 sampled (from a 9x replication of the tile) + the tile
itself; outputs overlap-add, then divide by overlap counts.

Algebraic collapse (verified ~1.3e-3 rel vs the jax reference on CPU):
  * rep[:, idx, :] == tile[:, idx % 64, :], so the sampled KV tokens are tile
    rows with integer multiplicities m_t[w] = 1 + #{s : idx[t,s] % 64 == w}.
  * Per-tile Q/K/V are slices of the full-sequence projections, so all
    per-tile 64x64 score blocks are diagonal blocks of one banded 512x512
    score matrix S = Q K^T (band |i-j| <= 63).
  * With E = exp(S - 20), Cm[j,t] = m_t[j-t] (banded), the full tile-softmax
    + overlap-add + count-divide collapses to
        Z = Cm^T E;  W = bandmask/(counts * Z);  U = Cm W;  out = (E*U)^T V
    computed entirely in the transposed [j, r] layout: S' = K Q^T is emitted
    directly by the PE (no E transposes), and a constant exp bias replaces
    the per-row max (softmax is shift-invariant; bf16/fp32 cover the range).
  * bk drops exactly: it scales each column's E and 1/Z by canceling factors.

Sharding: 8 cores = 2 batches x 4 row-chunks of 128 output rows. Each core
computes its 128 rows end-to-end from a 256-column band of the input (no
cross-core communication); host slices/pads inputs and concatenates outputs.

Schedule notes (all tuned against perfetto traces; 37.9us -> ~31us):
  * Everything ships fp16 (counts/Cm as bf16 bits: small ints, exact); all
    matmuls accumulate in fp32 PSUM. Score-path quantization error ~fp16 is
    ~2.5e-3 absolute on scores -> ~1.3e-3 output rel err, far under 2e-2.
  * A PSUM accumulation region must fully close (stop=True) before another
    region in the SAME bank issues start=True: start clears has_written
    bank-wide, silently dropping earlier regions' first contribution.
  * Input DMA: few LARGE pieces. An HWDGE ring leaves a ~2us gap between
    FIFO pieces (descriptors for piece N+1 wait on piece N's completion
    receipt), so 0.25MB pieces run a ring at ~1/3 duty cycle. Three queues
    (sync ring / scalar ring / gpsimd SWDGE) carry 2 pieces each, ordered
    by first use; the bias row ships as a single-partition 2.3KB piece
    instead of a mostly-zero [128, 1152] block.
  * ~8 throwaway warmup matmuls (>=3.6us busy) run while the first piece
    lands so the PE's HAM clock gate reaches 8/8 before the real stream.
  * PSUM->SBUF drains are split between Vector and Scalar (GPSIMD cannot
    read PSUM); reciprocal uses vector.reciprocal_approx_fast (~350ns vs
    ~1us for the exact DVE reciprocal; Z in [1e-13,1e11] is safe).
  * No-sync dependency edges pin the PE's static order to
    Z -> V(jc0) -> U -> V(jc1) -> out so V's twelve matmuls fill the PE
    idle windows while Vector/ACT run the recip/W and A stages.
  * The TileContext exit is instruction-free (bookkeeping only) and
    Bacc.reset()'s full-pool sem wipe is neutered: the walrus NEFF
    epilogue already barriers, drains, and sweeps every semaphore, so the
    in-body duplicates (~5us) are dead weight. Output-DMA completion is
    covered by that same ~8us epilogue (validated by double-execution).
"""

import os
import sys

import numpy as np

try:
    import ml_dtypes
except ImportError:
    ml_dtypes = None

for _p in ("/opt/trn_rl_repo",):
    if _p not in sys.path and os.path.isdir(_p):
        sys.path.insert(0, _p)

B, N, C, W = 2, 512, 512, 64
T = N - W + 1          # 449 tiles
RCH = 128              # output rows per core
NCORES = 8
BAND = 256             # per-core j/t band width (columns [r0-64, r0+192))
KC = C // 128          # 4 contraction chunks
JC = BAND // 128       # 2 band chunks
EXP_BIAS = -20.0       # constant softmax shift (shift-invariant; keeps
                       # exp args in a comfortably representable range)
N_WARM = 8             # PE warmup matmuls (HAM un-throttle)

# blob16 layout (2-byte elements per partition; fp16 except the bf16 Cm
# segments, which are bitcast views)
OFF_XT = 0                        # [128, 4, 256] fp16
OFF_WQT = OFF_XT + KC * BAND      # [128, 4, 512] fp16
OFF_WKT = OFF_WQT + KC * C        # [128, 4, 512] fp16
OFF_WVT = OFF_WKT + KC * C        # [128, 4, 512] fp16
OFF_CM = OFF_WVT + KC * C         # [128, 2, 256] bf16 bits
OFF_CMT = OFF_CM + JC * BAND      # [128, 2, 256] bf16 bits
OFF_MW = OFF_CMT + JC * BAND      # [128, 2, 128] fp16 (bandmask/counts)
OFF_MISC = OFF_MW + JC * RCH      # partition 0 only: bq[512] bv[512] ones[128]
MISC_LEN = 2 * C + 128
F16 = OFF_MISC + MISC_LEN

_CACHE = {}

WALRUS_MAX_SEM = 176   # caps walrus's own semaphore allocator; measured-best
                       # configuration shipped with this flag


def _install_walrus_flag_patch():
    """Append --max-sem-num to walrus_driver invocations. Idempotent,
    process-local."""
    import concourse.bass_utils as bu

    if getattr(bu, "_walrus_maxsem_patched", False):
        return
    orig = bu.run_command

    def patched(cmd, **kw):
        if (isinstance(cmd, list) and cmd
                and "walrus_driver" in str(cmd[0]) and "--pass" in cmd):
            cmd = list(cmd) + [f"--max-sem-num={WALRUS_MAX_SEM}"]
        return orig(cmd, **kw)

    bu.run_command = patched
    bu._walrus_maxsem_patched = True


def _slim_drain_and_barrier(self, tick_clock, wait_clock):
    """Instruction-free TileContext exit. The stock exit emits a global
    drain + barrier + sem clears + barrier (~3us); but this program's
    epilogue already contains Bacc.reset()'s two all-engine barriers and
    the walrus NEFF teardown (per-engine drains + a full semaphore sweep
    that zeroes every sem below 256), so everything the stock exit does is
    re-done later anyway. The output DMAs' completion receipt (~2us after
    last byte) lands well inside that ~8us epilogue, so nothing needs to
    block on the DMA clock either. Only the allocator bookkeeping stays."""
    popped = self.nc._tile_sem_poison_stack.pop()
    assert popped is self._sem_poison
    for h in self.sems.allocated().values():
        self.nc.release_semaphore(h)


def _build_program():
    import concourse.bacc as bacc
    import concourse.mybir as mybir
    import concourse.tile as tile

    _install_walrus_flag_patch()

    fp16 = mybir.dt.float16
    # Bass's preamble ends with a full all-engine barrier (drains + EVSEM,
    # ~3-5us with the PE's first-IRAM-block stall). Our kernel never reads
    # the preamble's const APs and all real cross-engine deps are Tile
    # semaphores, so skip it: engines start independently and the input DMA
    # issues ~5us earlier.
    orig_aeb = bacc.Bacc.all_engine_barrier

    def _noop_aeb(self, *, sem_only=False):
        return None

    bacc.Bacc.all_engine_barrier = _noop_aeb
    try:
        nc = bacc.Bacc("TRN2", target_bir_lowering=False, debug=False)
    finally:
        bacc.Bacc.all_engine_barrier = orig_aeb

    b16_d = nc.declare_dram_parameter("blob16", [128, F16], fp16, isOutput=False)
    out_d = nc.declare_dram_parameter("out", [RCH, C], mybir.dt.float32,
                                      isOutput=True)

    orig_dab = tile.TileContext._drain_and_barrier
    tile.TileContext._drain_and_barrier = _slim_drain_and_barrier
    try:
        _emit_body(nc, tile, mybir, b16_d, out_d)
    finally:
        tile.TileContext._drain_and_barrier = orig_dab

    # compile() emits Bacc.reset() -- the BSP re-entry block -- whose
    # gpsimd.sem_clear(range(3, 256)) lowers to ~250 per-sem EVENT_SEMAPHORE
    # resets spread over 5 engines (~7.5us of pure epilogue, inside the
    # measured window). Every sem this program ever increments is already
    # zeroed by the Tile drain (tile sems) or is self-balancing (barrier /
    # HWDGE-drain protocol sems), so the wipe is redundant: swap in a
    # gpsimd proxy that drops dma_reset/sem_clear during compile.
    class _GpsimdNoResetProxy:
        def __init__(self, real):
            object.__setattr__(self, "_real", real)

        def __getattr__(self, n):
            return getattr(self._real, n)

        def dma_reset(self, semaphore_range=None):
            return None

        def sem_clear(self, sem):
            return None

    real_gpsimd = nc.gpsimd
    nc.gpsimd = _GpsimdNoResetProxy(real_gpsimd)
    try:
        nc.compile()
    finally:
        nc.gpsimd = real_gpsimd
    return nc


def _emit_body(nc, tile, mybir, b16_d, out_d):
    fp32 = mybir.dt.float32
    fp16 = mybir.dt.float16
    bf16 = mybir.dt.bfloat16
    AF = mybir.ActivationFunctionType

    with tile.TileContext(nc) as tc:
        with (
            tc.tile_pool(name="consts", bufs=1) as consts,
            tc.tile_pool(name="work", bufs=1) as work,
            tc.tile_pool(name="psum", bufs=1, space="PSUM") as psum,
        ):
            b16 = consts.tile([128, F16], fp16)

            # ---- PE warmup: un-throttle HAM while the first DMAs land ----
            warm = work.tile([128, 512], fp16)
            nc.gpsimd.memset(warm[:], 0.0)
            ebias = work.tile([128, 1], fp32)
            nc.gpsimd.memset(ebias[:], EXP_BIAS)

            # ---- input DMA: few LARGE pieces over three queues (2 HWDGE
            # rings + 1 SWDGE). A ring leaves a ~2us gap between FIFO
            # pieces (next piece's descriptors wait for the previous
            # completion receipt), so many small pieces run the ring at
            # ~1/3 duty cycle; 1-2 big pieces per queue keep it streaming ----
            sync_pieces = [
                (OFF_WKT, OFF_WKT + KC * C),              # wkt     0.5MB
                (OFF_WVT, OFF_WVT + KC * C),              # wvt     0.5MB
            ]
            scalar_pieces = [
                (OFF_XT, OFF_WQT + 2 * C),                # xt+wqt m01 0.5MB
                (OFF_WQT + 2 * C, OFF_WQT + KC * C),      # wqt m23 0.25MB
            ]
            gpsimd_pieces = [
                None,                                     # misc (1 partition)
                (OFF_CM, OFF_MISC),                       # cm+cmt+mw 0.31MB
            ]
            for eng, pieces in ((nc.sync, sync_pieces),
                                (nc.scalar, scalar_pieces),
                                (nc.gpsimd, gpsimd_pieces)):
                for p in pieces:
                    if p is None:
                        eng.dma_start(
                            out=b16[0:1, OFF_MISC:OFF_MISC + MISC_LEN],
                            in_=b16_d[0:1, OFF_MISC:OFF_MISC + MISC_LEN],
                        )
                    else:
                        a, b = p
                        eng.dma_start(out=b16[:, a:b], in_=b16_d[:, a:b])

            xt_sb = b16[:, OFF_XT:OFF_XT + KC * BAND].rearrange(
                "p (k j) -> p k j", k=KC)
            # wq/wk ship m-major ([m][k][128] cols) so one DMA piece holds
            # two complete output chunks; wv stays k-major (full-N rhs)
            wqt_sb = b16[:, OFF_WQT:OFF_WQT + KC * C].rearrange(
                "p (m x) -> p m x", m=KC)
            wkt_sb = b16[:, OFF_WKT:OFF_WKT + KC * C].rearrange(
                "p (m x) -> p m x", m=KC)
            wvt_sb = b16[:, OFF_WVT:OFF_WVT + KC * C].rearrange(
                "p (k j) -> p k j", k=KC)
            cm_sb = b16[:, OFF_CM:OFF_CM + JC * BAND].bitcast(bf16).rearrange(
                "p (k t) -> p k t", k=JC)
            cmt_sb = b16[:, OFF_CMT:OFF_CMT + JC * BAND].bitcast(
                bf16).rearrange("p (k j) -> p k j", k=JC)
            mw_sb = b16[:, OFF_MW:OFF_MW + JC * RCH].rearrange(
                "p (k r) -> p k r", k=JC)
            bqr = b16[0:1, OFF_MISC:OFF_MISC + C]
            bvr = b16[0:1, OFF_MISC + C:OFF_MISC + 2 * C]
            ones1 = b16[0:1, OFF_MISC + 2 * C:OFF_MISC + 2 * C + 128]

            # PSUM plan (8 banks):
            #   qt [128,512] | kt [128,1024] | v [128,1024] | s [128,256]
            #   (S' jc0/jc1) | zu [128,512] (Z0 Z1 U0 U1) | o [128,512]
            #   (warmup matmuls park their dead results in o's first half)
            ps_qt = psum.tile([128, C], fp32, tag="ps_qt")
            ps_kt = psum.tile([128, KC * BAND], fp32, tag="ps_kt")
            ps_v = psum.tile([128, JC * C], fp32, tag="ps_v")
            ps_s = psum.tile([128, JC * RCH], fp32, tag="ps_s")
            ps_zu = psum.tile([128, 512], fp32, tag="ps_zu")
            ps_o = psum.tile([128, C], fp32, tag="ps_o")

            for i in range(N_WARM):
                nc.tensor.matmul(
                    ps_o[:], lhsT=warm[:, 0:128], rhs=warm[:],
                    start=True, stop=True,
                )

            # ---- QT[c,r] = Wq x_r^T + bq ----
            # NB: each accumulation region must fully close (stop=True)
            # before the next region's start=True in the same PSUM bank --
            # start clears has_written BANK-wide, so interleaving regions
            # silently drops the first contribution of all but the last.
            for m in range(KC):
                for k in range(KC):
                    nc.tensor.matmul(
                        ps_qt[:, m * 128:(m + 1) * 128],
                        lhsT=wqt_sb[:, m, k * 128:(k + 1) * 128],
                        rhs=xt_sb[:, k, 64:64 + RCH],
                        start=(k == 0), stop=False,
                    )
                nc.tensor.matmul(
                    ps_qt[:, m * 128:(m + 1) * 128],
                    lhsT=bqr[:, m * 128:(m + 1) * 128],
                    rhs=ones1, start=False, stop=True,
                )
            qt_sb = work.tile([128, KC * RCH], fp16)
            nc.vector.tensor_copy(out=qt_sb[:], in_=ps_qt)

            # ---- KT[c,j] = Wk x_band^T (bk dropped: softmax-invariant) ----
            for m in range(KC):
                for k in range(KC):
                    nc.tensor.matmul(
                        ps_kt[:, m * BAND:(m + 1) * BAND],
                        lhsT=wkt_sb[:, m, k * 128:(k + 1) * 128],
                        rhs=xt_sb[:, k, :],
                        start=(k == 0), stop=(k == KC - 1),
                    )
            kt_sb = work.tile([128, KC * BAND], fp16)
            nc.vector.tensor_copy(out=kt_sb[:, 0:512], in_=ps_kt[:, 0:512])
            nc.scalar.copy(out=kt_sb[:, 512:1024], in_=ps_kt[:, 512:1024])

            # ---- S'[j,r] = K Q^T, emitted transposed by the PE ----
            for jc in range(JC):
                for m in range(KC):
                    nc.tensor.matmul(
                        ps_s[:, jc * 128:(jc + 1) * 128],
                        lhsT=kt_sb[:, m * BAND + jc * 128:
                                   m * BAND + (jc + 1) * 128],
                        rhs=qt_sb[:, m * RCH:(m + 1) * RCH],
                        start=(m == 0), stop=(m == KC - 1),
                    )
            e_sb = work.tile([128, JC, RCH], bf16)
            for jc in range(JC):
                nc.scalar.activation(
                    out=e_sb[:, jc, :],
                    in_=ps_s[:, jc * 128:(jc + 1) * 128],
                    func=AF.Exp, bias=ebias[:], scale=1.0,
                )

            # ---- V[j,c] = x_band Wv^T + bv ----
            # V is off the critical path (only the final out matmuls read
            # it); no-sync edges below slot its two jc chunks into the PE
            # idle windows of the softmax chain (Z->recip/W and U->A).
            v_first = [None, None]
            v_last = [None, None]
            for jc in range(JC):
                for k in range(KC):
                    mmi = nc.tensor.matmul(
                        ps_v[:, jc * C:(jc + 1) * C],
                        lhsT=xt_sb[:, k, jc * 128:(jc + 1) * 128],
                        rhs=wvt_sb[:, k, :],
                        start=(k == 0), stop=False,
                    )
                    if k == 0:
                        v_first[jc] = mmi
                v_last[jc] = nc.tensor.matmul(
                    ps_v[:, jc * C:(jc + 1) * C],
                    lhsT=ones1, rhs=bvr, start=False, stop=True,
                )
            v_sb = work.tile([128, JC * C], fp16)
            nc.scalar.copy(out=v_sb[:, 0:C], in_=ps_v[:, 0:C])
            nc.scalar.copy(out=v_sb[:, C:2 * C], in_=ps_v[:, C:2 * C])

            # ---- Z[t,r] = Cm^T E;  W = maskw / Z ----
            w_sb = work.tile([128, JC, RCH], bf16)
            z_last = None
            for tch in range(JC):
                zslc = ps_zu[:, tch * 128:(tch + 1) * 128]
                for jc in range(JC):
                    z_last = nc.tensor.matmul(
                        zslc,
                        lhsT=cm_sb[:, jc, tch * 128:(tch + 1) * 128],
                        rhs=e_sb[:, jc, :],
                        start=(jc == 0), stop=(jc == JC - 1),
                    )
                r = work.tile([128, RCH], fp32, tag="rz", bufs=2)
                nc.vector.reciprocal_approx_fast(out=r[:], in_=zslc)
                nc.vector.tensor_mul(w_sb[:, tch, :], r, mw_sb[:, tch, :])

            # ---- U[j,r] = Cm W;  A = E * U ----
            a_sb = work.tile([128, JC, RCH], fp16)
            u_first = None
            u_last = None
            for jc in range(JC):
                uslc = ps_zu[:, 256 + jc * 128:256 + (jc + 1) * 128]
                for tch in range(JC):
                    mmi = nc.tensor.matmul(
                        uslc,
                        lhsT=cmt_sb[:, tch, jc * 128:(jc + 1) * 128],
                        rhs=w_sb[:, tch, :],
                        start=(tch == 0), stop=(tch == JC - 1),
                    )
                    if u_first is None:
                        u_first = mmi
                    u_last = mmi
                nc.vector.tensor_mul(a_sb[:, jc, :], uslc, e_sb[:, jc, :])

            # ---- out[r,c] = A^T V, two column halves overlap the DMA ----
            o_sb = work.tile([128, C], fp32)
            out_first = None
            for h in range(2):
                oslc = ps_o[:, h * 256:(h + 1) * 256]
                for jc in range(JC):
                    mmi = nc.tensor.matmul(
                        oslc,
                        lhsT=a_sb[:, jc, :],
                        rhs=v_sb[:, jc * C + h * 256:jc * C + (h + 1) * 256],
                        start=(jc == 0), stop=(jc == JC - 1),
                    )
                    if out_first is None:
                        out_first = mmi
                if h == 0:
                    nc.vector.tensor_copy(
                        out=o_sb[:, h * 256:(h + 1) * 256], in_=oslc)
                else:
                    nc.scalar.copy(
                        out=o_sb[:, h * 256:(h + 1) * 256], in_=oslc)
                eng = nc.sync if h == 0 else nc.scalar
                eng.dma_start(out=out_d[:, h * 256:(h + 1) * 256],
                              in_=o_sb[:, h * 256:(h + 1) * 256])

            # static PE order: Z -> V(jc0) -> U -> V(jc1) -> out, so V's
            # dozen matmuls fill the PE idle windows while the Vector/ACT
            # engines run the recip/W and A stages of the softmax chain
            from concourse.tile_rust import add_dep_helper
            add_dep_helper(v_first[0].ins, z_last.ins, False,
                           "static order: V jc0 after Z")
            add_dep_helper(u_first.ins, v_last[0].ins, False,
                           "static order: U after V jc0")
            add_dep_helper(v_first[1].ins, u_last.ins, False,
                           "static order: V jc1 after U")
            add_dep_helper(out_first.ins, v_last[1].ins, False,
                           "static order: out after V jc1")


def _pack128(arr):
    """[n*128, f] row-chunked -> [128, n*f] (chunk-major along free axis)."""
    n = arr.shape[0] // 128
    return np.ascontiguousarray(
        arr.reshape(n, 128, -1).transpose(1, 0, 2).reshape(128, -1)
    )


def _host_prep(image_features, Wq, bq, Wk, bk, Wv, bv, sample_idx):
    """Build the 8 per-core input blobs (pure index/layout work)."""
    x = np.asarray(image_features, np.float32)
    sample_idx = np.asarray(sample_idx)

    # per-tile multiplicities -> banded count matrix Cm[j, t] = m_t[j - t]
    mod = (sample_idx % W).astype(np.int64)                  # [T, S]
    m = np.zeros((T, W), np.float32)
    np.add.at(m, (np.arange(T)[:, None], mod), 1.0)
    m += 1.0
    Cm = np.zeros((N, N), np.float32)
    rows = np.arange(T)
    for w in range(W):
        Cm[rows + w, rows] = m[:, w]

    pos = np.arange(N)
    counts = (np.minimum(pos, N - W) - np.maximum(pos - W + 1, 0) + 1)

    # padded versions for uniform band slicing
    XTp = np.zeros((B, C, N + 2 * 64), np.float16)
    for b in range(B):
        XTp[b, :, 64:64 + N] = x[b].T.astype(np.float16)
    Cmp = np.zeros((N + 2 * 64, N + 2 * 64), np.float32)
    Cmp[64:64 + N, 64:64 + N] = Cm

    def _pack_mmajor(w):
        """Wq-like [cout, cin] -> [128, (m, k, 128)] columns: m-chunk-major
        so one contiguous DMA piece carries complete output chunks."""
        p = _pack128(np.asarray(w, np.float32).T.astype(np.float16))
        return np.ascontiguousarray(
            p.reshape(128, KC, KC, 128).transpose(0, 2, 1, 3).reshape(128, -1)
        )

    wqt_p = _pack_mmajor(Wq)
    wkt_p = _pack_mmajor(Wk)
    wvt_p = _pack128(np.asarray(Wv, np.float32).T.astype(np.float16))

    in_maps = []
    for core in range(NCORES):
        b, rc = divmod(core, NCORES // B)
        r0 = rc * RCH
        xt = XTp[b, :, r0:r0 + BAND]
        cm = np.ascontiguousarray(Cmp[r0:r0 + BAND, r0:r0 + BAND])
        # all-zero columns (padded t) would give Z=0 -> 1/0*mask = NaN on
        # device; a diagonal 1 keeps Z finite there and is masked out of W
        zero_cols = ~cm.any(axis=0)
        cm[zero_cols, zero_cols] = 1.0
        tl = np.arange(BAND)
        rl = np.arange(RCH)
        tg = r0 - 64 + tl
        rg = r0 + rl
        d = rg[None, :] - tg[:, None]
        valid = (d >= 0) & (d <= W - 1) & (tg[:, None] >= 0) & (tg[:, None] <= T - 1)
        maskw = np.where(
            valid, 1.0 / counts[rg][None, :], 0.0
        ).astype(np.float16)                                 # [t 256, r 128]

        b16 = np.zeros((128, F16), np.float16)
        b16[:, OFF_XT:OFF_XT + KC * BAND] = _pack128(xt)
        b16[:, OFF_WQT:OFF_WQT + KC * C] = wqt_p
        b16[:, OFF_WKT:OFF_WKT + KC * C] = wkt_p
        b16[:, OFF_WVT:OFF_WVT + KC * C] = wvt_p
        b16[:, OFF_MW:OFF_MW + JC * RCH] = _pack128(maskw)
        b16[0, OFF_MISC:OFF_MISC + C] = np.asarray(bq, np.float32)
        b16[0, OFF_MISC + C:OFF_MISC + 2 * C] = np.asarray(bv, np.float32)
        b16[0, OFF_MISC + 2 * C:OFF_MISC + 2 * C + 128] = 1.0
        # Cm segments carry bf16 bits (count ints are exact in bf16);
        # written through a uint16 view of the fp16 buffer
        b16v = b16.view(np.uint16)
        b16v[:, OFF_CM:OFF_CM + JC * BAND] = _pack128(
            cm.astype(ml_dtypes.bfloat16)).view(np.uint16)
        b16v[:, OFF_CMT:OFF_CMT + JC * BAND] = _pack128(
            np.ascontiguousarray(cm.T).astype(ml_dtypes.bfloat16)
        ).view(np.uint16)
        in_maps.append({"blob16": b16})
    return in_maps


def run_on_cores(in_maps, trace=False, trace_cores=None):
    from concourse.bass_utils import run_bass_kernel_spmd

    if "nc" not in _CACHE:
        _CACHE["nc"] = _build_program()
    nc = _CACHE["nc"]
    return run_bass_kernel_spmd(
        nc, in_maps, list(range(NCORES)), trace=trace,
        trace_cores=(trace_cores or [0]) if trace else None,
    )


def kernel(image_features, Wq, bq, Wk, bk, Wv, bv, sample_idx):
    in_maps = _host_prep(image_features, Wq, bq, Wk, bk, Wv, bv, sample_idx)
    res = run_on_cores(in_maps, trace=False)
    out = np.empty((B, N, C), np.float32)
    for core in range(NCORES):
        b, rc = divmod(core, NCORES // B)
        out[b, rc * RCH:(rc + 1) * RCH, :] = res.results[core]["out"]
    return out
